# revision 12
# baseline (speedup 1.0000x reference)
"""Trainium2 Bass kernel for a Mixtral decoder layer on 8 NeuronCores.

B=2,S=1024,H=2048, NH=16,NKV=4,HD=128, F=4096,E=8,TOPK=2.

Uniform SPMD program (identical BIR on all cores); per-core behavior is
carried by input data (weight shards, positions, mask codes, batch-select
beta, expert one-hot).

v3: attention chain kept at f32r precision (gating top-2 selection is
sensitive to ~1e-4 logit noise, so bf16 anywhere before the gate flips
experts), but all attention matmuls are 256-wide (f32r runs 1 cycle/row
at N>=256, same as bf16). QKV is sharded BY TOKENS (each core computes
all 24 column tiles for its own 2 chunks, streaming the full wqkv) which
removes the x AllGather and the qkv AllToAll entirely; only a KV-slice
AllGather remains. The MoE FFN (whose error does not feed the gate) runs
in bf16 with PSUM-accumulated w2 and a host-side scatter.

Pipeline per core:
  - rmsnorm + transpose of its two 128-token chunks -> xT_own (f32r).
  - QKV for own 256 tokens x all 3072 columns (wqkv streamed, f32r);
    RoPE (positions arrive as input data); K/V cols written + AllGather;
    Q transposed locally (no collective).
  - Causal attention for the 2 zigzag chunks as one 256-wide block per
    (head, key-chunk); out-proj (wo streamed f32r).
  - residual + rmsnorm; exact-f32 top-2 gating; AllGather probs (small,
    first) then x2 (bf16).
  - Expert-parallel MoE (expert c on core c), capacity C=640, gather via
    one-hot permutation matmuls (bf16); w1/w3/w2 bf16, w2 accumulated in
    PSUM; outputs scaled per-slot expert rows + rank vector; host
    scatters.
"""
import sys

sys.path.insert(0, "/opt/trn_rl_repo")
import math

import numpy as np
import ml_dtypes

import concourse.bass as bass
import concourse.mybir as mybir
import concourse.tile as tile
from concourse import bacc
from concourse.bass_utils import run_bass_kernel_spmd
from concourse.masks import make_identity

F32 = mybir.dt.float32
F32R = mybir.dt.float32r
BF16 = mybir.dt.bfloat16
AF = mybir.ActivationFunctionType
OP = mybir.AluOpType
AX = mybir.AxisListType

P = 128
B, S, H = 2, 1024, 2048
NH, NKV, HD = 16, 4, 128
F, E = 4096, 8
T = B * S
QKVW = (NH + 2 * NKV) * HD   # 3072
KVW = 2 * NKV * HD           # 1024
EPS = 1e-5
THETA = 10000.0
SCALE = 1.0 / math.sqrt(HD)
N_CORES = 8
C = 640                      # MoE capacity (observed max count 548)
CM = C // P                  # 5 capacity tiles
HKT = H // P                 # 16
CHS = S // P                 # 8 chunks per batch

# zigzag ownership: core c -> batch c//4, local chunks {j, 7-j}, j=c%4
OWN = [[(c // 4) * CHS + (c % 4), (c // 4) * CHS + (CHS - 1 - (c % 4))]
       for c in range(N_CORES)]
PCHUNKS = [g for c in range(N_CORES) for g in OWN[c]]
PIDX = {g: i for i, g in enumerate(PCHUNKS)}
PERM_TOKENS = np.concatenate([np.arange(g * P, (g + 1) * P) for g in PCHUNKS])

TWO_PI = 2.0 * math.pi
CW1 = 6.28125
CW2 = float(np.float32(TWO_PI - CW1))
CW3 = float(TWO_PI - CW1 - CW2)
INV2PI = 1.0 / TWO_PI


def build_nc(reps=1):
    nc = bacc.Bacc("TRN2", target_bir_lowering=False, debug=False,
                   num_devices=N_CORES)

    # ---------------- I/O ----------------
    hid_own = nc.dram_tensor("hid_own", [2 * P, H], F32, kind="ExternalInput")
    pos_t = nc.dram_tensor("pos_own", [P, 2], F32, kind="ExternalInput")
    wqkv_t = nc.dram_tensor("wqkv", [H, QKVW], F32, kind="ExternalInput")
    wo_t = nc.dram_tensor("wo", [NH * HD, H], F32, kind="ExternalInput")
    gate_t = nc.dram_tensor("gate_w", [H, E], F32, kind="ExternalInput")
    ln1_t = nc.dram_tensor("ln1_w", [H], F32, kind="ExternalInput")
    ln2_t = nc.dram_tensor("ln2_w", [H], F32, kind="ExternalInput")
    w1_t = nc.dram_tensor("w1_my", [H, F], BF16, kind="ExternalInput")
    w2_t = nc.dram_tensor("w2_my", [F, H], BF16, kind="ExternalInput")
    w3_t = nc.dram_tensor("w3_my", [H, F], BF16, kind="ExternalInput")
    maska_t = nc.dram_tensor("mask_a", [P, 2 * CHS], F32, kind="ExternalInput")
    maskb_t = nc.dram_tensor("mask_b", [P, 2 * CHS], F32, kind="ExternalInput")
    beta_t = nc.dram_tensor("beta", [P, 1], F32, kind="ExternalInput")
    onehot_t = nc.dram_tensor("onehot", [E, 1], F32, kind="ExternalInput")

    res2_own = nc.dram_tensor("res2_own", [2 * P, H], F32, kind="ExternalOutput")
    moe_cap = nc.dram_tensor("moe_cap", [C, H], F32, kind="ExternalOutput")
    rank_out = nc.dram_tensor("rank_out", [1, T], F32, kind="ExternalOutput")

    # collective buffers
    ag_kv_in = nc.dram_tensor("ag_kv_in", [2 * P, KVW], F32)
    ag_kv_out = nc.dram_tensor("ag_kv_out", [N_CORES * 2 * P, KVW], F32,
                               addr_space="Shared")
    PKG = E
    ag_p_in = nc.dram_tensor("ag_p_in", [PKG, 2 * P], F32)
    ag_p_out = nc.dram_tensor("ag_p_out", [N_CORES * PKG, 2 * P], F32,
                              addr_space="Shared")
    ag_n_in = nc.dram_tensor("ag_n_in", [2 * P, H], BF16)
    ag_n_out = nc.dram_tensor("ag_n_out", [T, H], BF16, addr_space="Shared")

    RG = [list(range(N_CORES))]

    with tile.TileContext(nc) as tc:
        with tc.tile_pool(name="singles", bufs=1) as singles:
            for _rep in range(reps):
                persist_cm = tc.tile_pool(name="persist", bufs=1)
                persist = persist_cm.__enter__()
                ident = singles.tile([P, P], F32)
                make_identity(nc, ident)
                # tri01[k,q] = 1 if k<=q else 0  (scoresT layout)
                tri01 = singles.tile([P, P], F32)
                nc.vector.memset(tri01, 1.0)
                nc.gpsimd.affine_select(out=tri01, in_=tri01, compare_op=OP.is_ge,
                                        fill=0.0, base=0, pattern=[[1, P]],
                                        channel_multiplier=-1)
                ones_colf = singles.tile([P, 1], F32)
                nc.vector.memset(ones_colf, 1.0)
                ones_col = singles.tile([P, 1], F32R)
                nc.vector.tensor_copy(ones_col, ones_colf)
                ln1_sb = singles.tile([P, HKT], F32)
                nc.sync.dma_start(ln1_sb, ln1_t.ap().rearrange("(kt p) -> p kt", p=P))
                ln2_row = singles.tile([1, H], F32)
                nc.sync.dma_start(ln2_row, ln2_t.ap().rearrange("(a h) -> a h", a=1))
                gw_sb = singles.tile([P, HKT, E], F32)
                nc.sync.dma_start(gw_sb, gate_t.ap().rearrange("(kt p) e -> p kt e", p=P))
                pos_sb = singles.tile([P, 2], F32)
                nc.sync.dma_start(pos_sb, pos_t.ap())
                maska = singles.tile([P, 2, CHS], F32)
                nc.sync.dma_start(maska, maska_t.ap().rearrange("p (s k) -> p s k", s=2))
                maskb = singles.tile([P, 2, CHS], F32)
                nc.sync.dma_start(maskb, maskb_t.ap().rearrange("p (s k) -> p s k", s=2))
                beta = singles.tile([P, 1], F32)
                nc.sync.dma_start(beta, beta_t.ap())
                ones_row1 = singles.tile([1, P], F32)
                nc.vector.memset(ones_row1, 1.0)
                eps_col = singles.tile([P, 1], F32)
                nc.vector.memset(eps_col, EPS)
                onehot = singles.tile([E, 1], F32R)
                nc.sync.dma_start(onehot, onehot_t.ap().bitcast(F32R))
                hid_sb = persist.tile([P, 2, H], F32)
                nc.sync.dma_start(hid_sb,
                                  hid_own.ap().rearrange("(ch p) h -> p ch h", p=P))
                ao_sb = persist.tile([P, 2, H], F32)
                qT = persist.tile([P, NH, 2, P], F32R)

                stagec_cm = tc.tile_pool(name="stagec", bufs=1)
                stc = stagec_cm.__enter__()
                xT_own = stc.tile([P, HKT, 2 * P], F32R)
                qkv_own = stc.tile([P, 2, QKVW], F32)
                cosT = stc.tile([P, 2, 64], F32)
                sinT = stc.tile([P, 2, 64], F32)

                # rope tables for own 2 chunks (positions are input data)
                with tc.tile_pool(name="ropetmp", bufs=1) as rtmp, \
                     tc.tile_pool(name="psrope", bufs=1, space="PSUM") as psrope:
                    invf_row = rtmp.tile([1, 64], F32)
                    nc.gpsimd.iota(invf_row, pattern=[[1, 64]], base=0,
                                   channel_multiplier=0,
                                   allow_small_or_imprecise_dtypes=True)
                    nc.scalar.activation(invf_row, invf_row, AF.Exp,
                                         scale=-math.log(THETA) / 64.0)
                    pibc = psrope.tile([P, 64], F32)
                    nc.tensor.matmul(pibc, ones_row1, invf_row, start=True, stop=True)
                    invf_bc = rtmp.tile([P, 64], F32)
                    nc.vector.tensor_copy(invf_bc, pibc)
                    ang = rtmp.tile([P, 2, 64], F32)
                    nc.vector.tensor_tensor(
                        ang, pos_sb[:, :, None].to_broadcast([P, 2, 64]),
                        invf_bc[:, None, :].to_broadcast([P, 2, 64]),
                        op=OP.mult)
                    x_t = rtmp.tile([P, 2, 64], F32)
                    nc.vector.tensor_scalar_mul(x_t, ang, INV2PI)
                    ki32 = rtmp.tile([P, 2, 64], mybir.dt.int32)
                    nc.vector.tensor_copy(ki32, x_t)
                    nc.vector.tensor_copy(x_t, ki32)
                    y_t = rtmp.tile([P, 2, 64], F32)
                    fl = "p a b -> p (a b)"
                    nc.vector.cody_waite_cascade(y_t.rearrange(fl),
                                                 ang.rearrange(fl),
                                                 x_t.rearrange(fl),
                                                 CW1, CW2, CW3)
                    ys = rtmp.tile([P, 2, 64], F32)
                    nc.vector.add_range_wrap(ys.rearrange(fl), y_t.rearrange(fl),
                                             0.0, math.pi, TWO_PI)
                    nc.scalar.activation(sinT, ys, AF.Sin)
                    nc.vector.add_range_wrap(ys.rearrange(fl), y_t.rearrange(fl),
                                             math.pi / 2.0, math.pi, TWO_PI)
                    nc.scalar.activation(cosT, ys, AF.Sin)

                # ========== Stage B: own-chunk rmsnorm + transpose ==========
                with tc.tile_pool(name="stageb", bufs=2) as stb, \
                     tc.tile_pool(name="psb", bufs=2, space="PSUM") as psb:
                    ssq = stb.tile([P, 2], F32)
                    scr = stb.tile([P, H], F32, tag="scr")
                    for ch in range(2):
                        nc.scalar.activation(scr, hid_sb[:, ch, :], AF.Square,
                                             accum_out=ssq[:, ch:ch + 1])
                    s_sc = stb.tile([P, 2], F32)
                    nc.scalar.activation(s_sc, ssq, AF.Sqrt, bias=eps_col, scale=1.0 / H)
                    nc.vector.reciprocal(s_sc, s_sc)
                    hsc = stb.tile([P, 2, H], F32)
                    for ch in range(2):
                        nc.scalar.activation(hsc[:, ch, :], hid_sb[:, ch, :],
                                             AF.Copy, scale=s_sc[:, ch:ch + 1])
                    for kt in range(HKT):
                        for ch in range(2):
                            ps = psb.tile([P, P], F32, tag="ps")
                            nc.tensor.transpose(ps, hsc[:, ch, kt * P:(kt + 1) * P],
                                                ident)
                            nc.vector.tensor_scalar(
                                xT_own[:, kt, ch * P:(ch + 1) * P], ps,
                                ln1_sb[:, kt:kt + 1], None, OP.mult)

                # ====== Stage C: QKV for own tokens (wqkv streamed) ======
                with tc.tile_pool(name="wqst", bufs=3) as wqst, \
                     tc.tile_pool(name="psc", bufs=1, space="PSUM") as psc, \
                     tc.tile_pool(name="stc2", bufs=2) as stc2:
                    for cf in range(2):
                        pq = [psc.tile([P, 512], F32, tag=f"pq{n}", name=f"pq{n}")
                              for n in range(6)]
                        for kt in range(HKT):
                            wq_kt = wqst.tile([P, QKVW // 2], F32R, tag="wq")
                            nc.sync.dma_start(
                                wq_kt,
                                wqkv_t.ap().bitcast(F32R)[
                                    kt * P:(kt + 1) * P,
                                    cf * (QKVW // 2):(cf + 1) * (QKVW // 2)])
                            for ch in range(2):
                                for n in range(3):
                                    nc.tensor.matmul(
                                        pq[ch * 3 + n],
                                        xT_own[:, kt, ch * P:(ch + 1) * P],
                                        wq_kt[:, n * 512:(n + 1) * 512],
                                        start=(kt == 0), stop=(kt == HKT - 1))
                        for ch in range(2):
                            for n in range(3):
                                nc.vector.tensor_copy(
                                    qkv_own[:, ch,
                                            cf * (QKVW // 2) + n * 512:
                                            cf * (QKVW // 2) + (n + 1) * 512],
                                    pq[ch * 3 + n])

                    # rope helper on [P, 2, 64] half-pairs
                    rt1 = stc2.tile([P, 2, 64], F32, tag="rt1")
                    rt2 = stc2.tile([P, 2, 64], F32, tag="rt2")
                    rtb = stc2.tile([P, 2, 64], F32, tag="rtb")

                    def rope_m(m):
                        x1 = qkv_own[:, :, m * P: m * P + 64]
                        x2_ = qkv_own[:, :, m * P + 64: (m + 1) * P]
                        nc.vector.tensor_mul(rt1, x1, cosT)
                        nc.vector.tensor_mul(rtb, x2_, sinT)
                        nc.vector.tensor_sub(rt1, rt1, rtb)
                        nc.vector.tensor_mul(rt2, x1, sinT)
                        nc.vector.tensor_mul(rtb, x2_, cosT)
                        nc.vector.tensor_add(rt2, rt2, rtb)
                        nc.vector.tensor_copy(x1, rt1)
                        nc.vector.tensor_copy(x2_, rt2)

                    # K tiles first so the KV AllGather launches earliest
                    for m in range(NH, NH + NKV):
                        rope_m(m)
                    for ch in range(2):
                        nc.sync.dma_start(
                            ag_kv_in.ap()[ch * P:(ch + 1) * P, :],
                            qkv_own[:, ch, NH * HD:])
                    nc.gpsimd.collective_compute(
                        "AllGather", OP.bypass, ins=[ag_kv_in.ap()],
                        outs=[ag_kv_out.ap()], replica_groups=RG)
                    for m in range(NH):
                        rope_m(m)
                # q transposed into qT (persist pool) before qkv_own frees
                with tc.tile_pool(name="qtp", bufs=3, space="PSUM") as qtp:
                    for h in range(NH):
                        for ch in range(2):
                            pq2 = qtp.tile([P, P], F32, tag="pq2")
                            nc.tensor.transpose(
                                pq2, qkv_own[:, ch, h * P:(h + 1) * P], ident)
                            nc.vector.tensor_copy(qT[:, h, ch, :], pq2)
                stagec_cm.__exit__(None, None, None)

                # ================= Stage D: attention =====================
                with tc.tile_pool(name="staged", bufs=1) as std_:
                    # mask01[p, kc, s, k]
                    mask01 = std_.tile([P, CHS, 2, P], F32)
                    for kc in range(CHS):
                        for s in range(2):
                            nc.vector.scalar_tensor_tensor(
                                mask01[:, kc, s, :], tri01, maskb[:, s, kc:kc + 1],
                                maska[:, s, kc:kc + 1].to_broadcast([P, P]),
                                op0=OP.mult, op1=OP.add)
                    ombeta = std_.tile([P, 1], F32)
                    nc.vector.tensor_scalar(ombeta, beta, -1.0, 1.0, OP.mult, OP.add)
                    kT = std_.tile([P, NKV, CHS, P], F32R)
                    vn = std_.tile([P, NKV, CHS, P], F32R)
                    with tc.tile_pool(name="ldd", bufs=3) as ldd, \
                         tc.tile_pool(name="psl", bufs=3, space="PSUM") as psl:
                        for kv in range(NKV):
                            for kc in range(CHS):
                                kb = [None, None]
                                vb = [None, None]
                                for b in range(2):
                                    row = PIDX[b * CHS + kc] * P
                                    kb[b] = ldd.tile([P, P], F32, tag=f"kb{b}",
                                                     name=f"kb{b}")
                                    nc.sync.dma_start(
                                        kb[b], ag_kv_out.ap()[row:row + P,
                                                              kv * P:(kv + 1) * P])
                                    vb[b] = ldd.tile([P, P], F32, tag=f"vb{b}",
                                                     name=f"vb{b}")
                                    nc.sync.dma_start(
                                        vb[b], ag_kv_out.ap()[
                                            row:row + P,
                                            (NKV + kv) * P:(NKV + kv + 1) * P])
                                # blend: use = b0*(1-beta) + b1*beta
                                kn = ldd.tile([P, P], F32, tag="kn")
                                nc.vector.tensor_scalar(kb[0], kb[0], ombeta, None,
                                                        OP.mult)
                                nc.vector.scalar_tensor_tensor(
                                    kn, kb[1], beta, kb[0],
                                    op0=OP.mult, op1=OP.add)
                                pk2 = psl.tile([P, P], F32, tag="pk2")
                                nc.tensor.transpose(pk2, kn, ident)
                                nc.vector.tensor_copy(kT[:, kv, kc, :], pk2)
                                nc.vector.tensor_scalar(vb[0], vb[0], ombeta, None,
                                                        OP.mult)
                                nc.vector.scalar_tensor_tensor(
                                    vn[:, kv, kc, :], vb[1], beta,
                                    vb[0], op0=OP.mult, op1=OP.add)
                    avT = std_.tile([P, NH, 2, P], F32R)
                    with tc.tile_pool(name="expp", bufs=4) as expp, \
                         tc.tile_pool(name="psa", bufs=3, space="PSUM") as psa, \
                         tc.tile_pool(name="psa2", bufs=2, space="PSUM") as psa2, \
                         tc.tile_pool(name="psa3", bufs=2, space="PSUM") as psa3:
                        for h in range(NH):
                            kv = h // (NH // NKV)
                            qh = qT[:, h, :, :].rearrange("p s d -> p (s d)")
                            pav = psa2.tile([P, 2 * P], F32, tag="pav")
                            pse = psa3.tile([1, 2 * P], F32, tag="pse")
                            for kc in range(CHS):
                                pss = psa.tile([P, 2 * P], F32, tag="pss")
                                nc.tensor.matmul(pss, kT[:, kv, kc, :], qh,
                                                 start=True, stop=True)
                                ex = expp.tile([P, 2 * P], F32, tag="ex")
                                nc.scalar.activation(ex, pss, AF.Exp, scale=SCALE)
                                exm = expp.tile([P, 2 * P], F32R, tag="exm")
                                nc.vector.tensor_mul(
                                    exm, ex,
                                    mask01[:, kc, :, :].rearrange("p s k -> p (s k)"))
                                nc.tensor.matmul(pse, ones_col, exm,
                                                 start=(kc == 0),
                                                 stop=(kc == CHS - 1))
                                nc.tensor.matmul(pav, vn[:, kv, kc, :], exm,
                                                 start=(kc == 0),
                                                 stop=(kc == CHS - 1))
                            rden = expp.tile([1, 2 * P], F32, tag="rden")
                            nc.vector.reciprocal(rden, pse)
                            prb = psa.tile([P, 2 * P], F32, tag="pss")
                            nc.tensor.matmul(prb, ones_row1, rden,
                                             start=True, stop=True)
                            rb_sb = expp.tile([P, 2 * P], F32, tag="rb_sb")
                            nc.vector.tensor_copy(rb_sb, prb)
                            nc.vector.tensor_mul(
                                avT[:, h, :, :].rearrange("p s d -> p (s d)"),
                                pav, rb_sb)
                    with tc.tile_pool(name="wop", bufs=2) as wop, \
                         tc.tile_pool(name="pso", bufs=3, space="PSUM") as pso:
                        wor = wo_t.ap().bitcast(F32R).rearrange(
                            "(kt p) h -> p kt h", p=P)
                        for n in range(8):
                            wo_n = wop.tile([P, NH, 256], F32R, tag="wo")
                            nc.sync.dma_start(wo_n, wor[:, :, n * 256:(n + 1) * 256])
                            for s in range(2):
                                po = pso.tile([P, 256], F32, tag="po")
                                for h in range(NH):
                                    nc.tensor.matmul(po, avT[:, h, s, :],
                                                     wo_n[:, h, :],
                                                     start=(h == 0),
                                                     stop=(h == NH - 1))
                                nc.vector.tensor_copy(
                                    ao_sb[:, s, n * 256:(n + 1) * 256], po)

                # q transposes moved into stage D via qT fill (see below)

                # ============ Stage E: residual2, rmsnorm, gating ===========
                with tc.tile_pool(name="stagee", bufs=1) as ste, \
                     tc.tile_pool(name="ste2", bufs=3) as ste2, \
                     tc.tile_pool(name="psg", bufs=1, space="PSUM") as psg, \
                     tc.tile_pool(name="pse_", bufs=2, space="PSUM") as pse_:
                    res2 = ste.tile([P, 2, H], F32)
                    nc.vector.tensor_add(res2, ao_sb, hid_sb)
                    nc.sync.dma_start(
                        res2_own.ap().rearrange("(ch p) h -> p ch h", p=P), res2)
                    ssq2 = ste.tile([P, 2], F32)
                    scr2 = ste.tile([P, H], F32, tag="scr2")
                    for ch in range(2):
                        nc.scalar.activation(scr2, res2[:, ch, :], AF.Square,
                                             accum_out=ssq2[:, ch:ch + 1])
                    s2 = ste.tile([P, 2], F32)
                    nc.scalar.activation(s2, ssq2, AF.Sqrt, bias=eps_col[:, :], scale=1.0 / H)
                    nc.vector.reciprocal(s2, s2)
                    ln2_bc = ste.tile([P, H], F32)
                    for n in range(4):
                        pl2 = pse_.tile([P, 512], F32, tag="pl2")
                        nc.tensor.matmul(pl2, ones_row1,
                                         ln2_row[:, n * 512:(n + 1) * 512],
                                         start=True, stop=True)
                        nc.vector.tensor_copy(ln2_bc[:, n * 512:(n + 1) * 512], pl2)
                    x2 = ste.tile([P, 2, H], F32)
                    for ch in range(2):
                        nc.scalar.activation(x2[:, ch, :], res2[:, ch, :], AF.Copy,
                                             scale=s2[:, ch:ch + 1])
                    nc.vector.tensor_mul(
                        x2, x2, ln2_bc[:, None, :].to_broadcast([P, 2, H]))
                    x2_bf = ste.tile([P, 2, H], BF16)
                    nc.vector.tensor_copy(x2_bf, x2)
                    nc.sync.dma_start(
                        ag_n_in.ap().rearrange("(ch p) h -> p ch h", p=P), x2_bf)
                    pg = [psg.tile([P, E], F32, tag=f"pg{ch}", name=f"pg{ch}") for ch in range(2)]
                    for kt in range(HKT):
                        for ch in range(2):
                            pt2 = pse_.tile([P, P], F32, tag="pt2")
                            nc.tensor.transpose(pt2, x2[:, ch, kt * P:(kt + 1) * P],
                                                ident)
                            x2t = ste2.tile([P, P], F32, tag="x2t")
                            nc.vector.tensor_copy(x2t, pt2)
                            nc.tensor.matmul(pg[ch], x2t, gw_sb[:, kt, :],
                                             start=(kt == 0), stop=(kt == HKT - 1))
                    for ch in range(2):
                        m1 = ste2.tile([P, 1], F32, tag="m1")
                        nc.vector.reduce_max(m1, pg[ch], axis=AX.X)
                        nm1 = ste2.tile([P, 1], F32, tag="nm1")
                        nc.vector.tensor_scalar_mul(nm1, m1, -1.0)
                        ee = ste2.tile([P, E], F32, tag="ee")
                        nc.scalar.activation(ee, pg[ch], AF.Exp, bias=nm1)
                        eq1 = ste2.tile([P, E], F32, tag="eq1")
                        nc.vector.tensor_scalar(eq1, ee, 1.0, None, OP.is_ge)
                        e2in = ste2.tile([P, E], F32, tag="e2in")
                        nc.vector.scalar_tensor_tensor(e2in, eq1, -2.0, ee,
                                                       op0=OP.mult, op1=OP.add)
                        e2 = ste2.tile([P, 1], F32, tag="e2")
                        nc.vector.reduce_max(e2, e2in, axis=AX.X)
                        den = ste2.tile([P, 1], F32, tag="den")
                        nc.vector.tensor_scalar_add(den, e2, 1.0)
                        rden2 = ste2.tile([P, 1], F32, tag="rden2")
                        nc.vector.reciprocal(rden2, den)
                        sel = ste2.tile([P, E], F32, tag="sel")
                        nc.vector.tensor_tensor(sel, ee, e2.to_broadcast([P, E]),
                                                op=OP.is_ge)
                        ww = ste2.tile([P, E], F32, tag="ww")
                        nc.vector.tensor_mul(ww, ee, sel)
                        nc.scalar.activation(ww, ww, AF.Copy, scale=rden2)
                        pw = pse_.tile([E, P], F32, tag="pw")
                        nc.tensor.transpose(pw, ww, ident)
                        wt_sb = ste2.tile([E, P], F32, tag="wt")
                        nc.vector.tensor_copy(wt_sb, pw)
                        nc.sync.dma_start(
                            ag_p_in.ap()[0:E, ch * P:(ch + 1) * P], wt_sb)
                # probs first (small; rank compute overlaps the x2 AG)
                nc.gpsimd.collective_compute(
                    "AllGather", OP.bypass, ins=[ag_p_in.ap()],
                    outs=[ag_p_out.ap()], replica_groups=RG)
                nc.gpsimd.collective_compute(
                    "AllGather", OP.bypass, ins=[ag_n_in.ap()],
                    outs=[ag_n_out.ap()], replica_groups=RG)

                persist_cm.__exit__(None, None, None)
                # ========== Stage F: routed MoE (expert = one-hot input) =====
                NCH = [(0, 512), (512, C - 512)]
                with tc.tile_pool(name="moeA", bufs=1) as moeA:
                    w_row = moeA.tile([1, T], F32)
                    rank_m = moeA.tile([1, T], F32)
                    rank_col = moeA.tile([P, HKT], F32)
                    w_col = moeA.tile([P, HKT], F32R)
                    with tc.tile_pool(name="moeR", bufs=1) as moeR, \
                         tc.tile_pool(name="psr", bufs=4, space="PSUM") as psr:
                        w_all = moeR.tile([E, T], F32R)
                        for cc in range(N_CORES):
                            nc.sync.dma_start(
                                w_all[:, cc * 2 * P:(cc + 1) * 2 * P],
                                ag_p_out.ap().bitcast(F32R)[
                                    cc * PKG: cc * PKG + E, :])
                        for n in range(4):
                            pwr = psr.tile([1, 512], F32, tag="pf")
                            nc.tensor.matmul(pwr, onehot,
                                             w_all[:, n * 512:(n + 1) * 512],
                                             start=True, stop=True)
                            nc.vector.tensor_copy(w_row[:, n * 512:(n + 1) * 512],
                                                  pwr)
                        sel_row = moeR.tile([1, T], F32)
                        nc.vector.tensor_scalar(sel_row, w_row, 0.0, None, OP.is_gt)
                        zeros_row = moeR.tile([1, T], F32)
                        nc.vector.memset(zeros_row, 0.0)
                        rank_row = moeR.tile([1, T], F32)
                        nc.vector.tensor_tensor_scan(rank_row, sel_row, zeros_row,
                                                     0.0, op0=OP.add, op1=OP.add)
                        nc.vector.scalar_tensor_tensor(rank_m, rank_row, 1.0,
                                                       sel_row, op0=OP.add,
                                                       op1=OP.mult)
                        nc.vector.tensor_scalar_add(rank_m, rank_m, -1.0)
                        nc.sync.dma_start(rank_out.ap(), rank_m)
                        for kt in range(HKT):
                            prc = psr.tile([P, 1], F32, tag="pf")
                            nc.tensor.transpose(prc, rank_m[:, kt * P:(kt + 1) * P],
                                                ident[:1, :1])
                            nc.vector.tensor_copy(rank_col[:, kt:kt + 1], prc)
                            pwc = psr.tile([P, 1], F32, tag="pf")
                            nc.tensor.transpose(pwc, w_row[:, kt * P:(kt + 1) * P],
                                                ident[:1, :1])
                            nc.vector.tensor_copy(w_col[:, kt:kt + 1], pwc)

                    w_g = moeA.tile([P, CM], F32)
                    xgT = moeA.tile([P, HKT, C], BF16)
                    with tc.tile_pool(name="moeB", bufs=1) as moeB:
                        iotaC_bc = moeB.tile([P, C], F32)
                        nc.gpsimd.iota(iotaC_bc, pattern=[[1, C]], base=1,
                                       channel_multiplier=0,
                                       allow_small_or_imprecise_dtypes=True)
                        with tc.tile_pool(name="ptpool", bufs=1) as ptp, \
                             tc.tile_pool(name="xnst", bufs=2) as xnst, \
                             tc.tile_pool(name="pswg", bufs=1, space="PSUM") as pswg, \
                             tc.tile_pool(name="psx", bufs=4, space="PSUM") as psx:
                            PTf = ptp.tile([P, C], F32R, tag="ptf")
                            PT = ptp.tile([P, HKT, C], BF16)
                            pwg = pswg.tile([1, 512], F32, tag="pwg")
                            pwgs = pswg.tile([1, P], F32, tag="pwgs")
                            for kt in range(HKT):
                                nc.vector.tensor_tensor(
                                    PTf,
                                    rank_col[:, kt:kt + 1].to_broadcast([P, C]),
                                    iotaC_bc, op=OP.is_equal)
                                nc.vector.tensor_copy(PT[:, kt, :], PTf)
                                nc.tensor.matmul(pwg, w_col[:, kt:kt + 1],
                                                 PTf[:, 0:512],
                                                 start=(kt == 0),
                                                 stop=(kt == HKT - 1))
                                nc.tensor.matmul(pwgs, w_col[:, kt:kt + 1],
                                                 PTf[:, 512:C],
                                                 start=(kt == 0),
                                                 stop=(kt == HKT - 1))
                            wgrow = moeB.tile([1, C], F32)
                            nc.vector.tensor_copy(wgrow[:, 0:512], pwg)
                            nc.vector.tensor_copy(wgrow[:, 512:C], pwgs)
                            for cm in range(CM):
                                pwg2 = psx.tile([P, 1], F32, tag="px")
                                nc.tensor.transpose(
                                    pwg2, wgrow[:, cm * P:(cm + 1) * P],
                                    ident[:1, :1])
                                nc.vector.tensor_copy(w_g[:, cm:cm + 1], pwg2)
                            for m in range(HKT):
                                xn = xnst.tile([P, HKT, P], BF16, tag="xn")
                                for kt in range(HKT):
                                    nc.sync.dma_start(
                                        xn[:, kt, :],
                                        ag_n_out.ap()[
                                            kt * P:(kt + 1) * P, m * P:(m + 1) * P])
                                for (n0, nw) in NCH:
                                    px = psx.tile([P, 512], F32, tag="px")
                                    for kt in range(HKT):
                                        nc.tensor.matmul(
                                            px[:, :nw], xn[:, kt, :],
                                            PT[:, kt, n0:n0 + nw],
                                            start=(kt == 0), stop=(kt == HKT - 1))
                                    nc.vector.tensor_copy(
                                        xgT[:, m, n0:n0 + nw], px[:, :nw])

                    # expert FFN over capacity slots: phase A builds all of
                    # act = silu(w1x) * w3x in bf16; phase B runs w2 with
                    # full PSUM accumulation (no SBUF adds).
                    act = moeA.tile([P, 32, C], BF16)
                    with tc.tile_pool(name="wstream", bufs=4) as wst, \
                         tc.tile_pool(name="psh", bufs=4, space="PSUM") as psh, \
                         tc.tile_pool(name="psh2", bufs=4, space="PSUM") as psh2:
                        w1r = w1_t.ap().rearrange("(kt p) f -> p kt f", p=P)
                        w3r = w3_t.ap().rearrange("(kt p) f -> p kt f", p=P)
                        for fs in range(8):
                            for half in range(2):
                                ph1 = [psh.tile([P, 512], F32, tag="ph512", name="ph1")
                                       for _ in range(2)]
                                ph1s = [psh2.tile([P, P], F32, tag="ph128", name="ph1s")
                                        for _ in range(2)]
                                ph3 = [psh.tile([P, 512], F32, tag="ph512", name="ph3")
                                       for _ in range(2)]
                                ph3s = [psh2.tile([P, P], F32, tag="ph128", name="ph3s")
                                        for _ in range(2)]
                                col0 = fs * 512 + half * 256
                                for kt in range(HKT):
                                    w1k = wst.tile([P, 256], BF16, tag="w1k")
                                    nc.sync.dma_start(
                                        w1k, w1r[:, kt, col0:col0 + 256])
                                    w3k = wst.tile([P, 256], BF16, tag="w3k")
                                    nc.sync.dma_start(
                                        w3k, w3r[:, kt, col0:col0 + 256])
                                    first, last = kt == 0, kt == HKT - 1
                                    for ms in range(2):
                                        for (n0, nw) in NCH:
                                            pt_ = (ph1 if nw == 512 else ph1s)[ms]
                                            nc.tensor.matmul(
                                                pt_[:, :nw],
                                                w1k[:, ms * P:(ms + 1) * P],
                                                xgT[:, kt, n0:n0 + nw],
                                                start=first, stop=last)
                                            pt3 = (ph3 if nw == 512 else ph3s)[ms]
                                            nc.tensor.matmul(
                                                pt3[:, :nw],
                                                w3k[:, ms * P:(ms + 1) * P],
                                                xgT[:, kt, n0:n0 + nw],
                                                start=first, stop=last)
                                for ms in range(2):
                                    fslot = fs * 4 + half * 2 + ms
                                    for (n0, nw) in NCH:
                                        p1 = (ph1 if nw == 512 else ph1s)[ms]
                                        p3 = (ph3 if nw == 512 else ph3s)[ms]
                                        sl = wst.tile([P, 512], F32, tag="silu")
                                        nc.scalar.activation(
                                            sl[:, :nw], p1[:, :nw], AF.Silu)
                                        nc.vector.tensor_mul(
                                            act[:, fslot, n0:n0 + nw],
                                            sl[:, :nw], p3[:, :nw])

                    with tc.tile_pool(name="w2stream", bufs=2) as w2st, \
                         tc.tile_pool(name="scst", bufs=3) as scst, \
                         tc.tile_pool(name="psw2", bufs=2, space="PSUM") as psw2:
                        w2r = w2_t.ap().rearrange("(sl p) h -> p sl h", p=P)
                        for n in range(4):
                            w2n = w2st.tile([P, 32, 512], BF16, tag="w2n")
                            nc.sync.dma_start(
                                w2n, w2r[:, :, n * 512:(n + 1) * 512])
                            for cm in range(CM):
                                peo = psw2.tile([P, 512], F32, tag="peo")
                                for slot in range(32):
                                    nc.tensor.matmul(
                                        peo,
                                        act[:, slot, cm * P:(cm + 1) * P],
                                        w2n[:, slot, :],
                                        start=(slot == 0), stop=(slot == 31))
                                ob = scst.tile([P, 512], F32, tag="ob")
                                nc.scalar.activation(ob, peo, AF.Copy,
                                                     scale=w_g[:, cm:cm + 1])
                                nc.sync.dma_start(
                                    moe_cap.ap()[cm * P:(cm + 1) * P,
                                                 n * 512:(n + 1) * 512], ob)

    nc.compile()
    return nc


_NC = None


def _get_nc():
    global _NC
    if _NC is None:
        _NC = build_nc()
    return _NC


def _prepare_in_maps(inputs):
    bf = ml_dtypes.bfloat16
    hs = np.asarray(inputs["hidden_states"], np.float32).reshape(T, H)
    wqkv = np.ascontiguousarray(np.asarray(inputs["wqkv"], np.float32))
    wo = np.ascontiguousarray(np.asarray(inputs["wo"], np.float32))
    gate_w = np.ascontiguousarray(np.asarray(inputs["gate_w"], np.float32))
    ln1 = np.asarray(inputs["ln1_w"], np.float32)
    ln2 = np.asarray(inputs["ln2_w"], np.float32)
    w1 = np.asarray(inputs["w1"], np.float32)
    w2 = np.asarray(inputs["w2"], np.float32)
    w3 = np.asarray(inputs["w3"], np.float32)
    in_maps = []
    for c in range(N_CORES):
        g0, g1 = OWN[c]
        hid_own = np.concatenate([hs[g0 * P:(g0 + 1) * P],
                                  hs[g1 * P:(g1 + 1) * P]], 0)
        pos = np.stack([np.arange(P, dtype=np.float32) + (g0 % CHS) * P,
                        np.arange(P, dtype=np.float32) + (g1 % CHS) * P], 1)
        j = c % 4
        own_local = [j, CHS - 1 - j]
        mask_a = np.zeros((P, 2, CHS), np.float32)
        mask_b = np.zeros((P, 2, CHS), np.float32)
        for s in range(2):
            jq = own_local[s]
            for kc in range(CHS):
                if kc < jq:
                    mask_a[:, s, kc] = 1.0
                elif kc == jq:
                    mask_b[:, s, kc] = 1.0
        beta = np.full((P, 1), float(c // 4), np.float32)
        onehot = np.zeros((E, 1), np.float32)
        onehot[c] = 1.0
        in_maps.append({
            "hid_own": np.ascontiguousarray(hid_own),
            "pos_own": pos,
            "wqkv": wqkv,
            "wo": wo,
            "gate_w": gate_w,
            "ln1_w": ln1,
            "ln2_w": ln2,
            "w1_my": np.ascontiguousarray(w1[c]).astype(bf),
            "w2_my": np.ascontiguousarray(w2[c]).astype(bf),
            "w3_my": np.ascontiguousarray(w3[c]).astype(bf),
            "mask_a": mask_a.reshape(P, 2 * CHS),
            "mask_b": mask_b.reshape(P, 2 * CHS),
            "beta": beta,
            "onehot": onehot,
        })
    return in_maps


def kernel(**inputs):
    nc = _get_nc()
    in_maps = _prepare_in_maps(inputs)
    res = run_bass_kernel_spmd(nc, in_maps, core_ids=list(range(N_CORES)))
    results = res.results
    moe = np.zeros((T, H), np.float32)
    res2 = np.zeros((T, H), np.float32)
    for c in range(N_CORES):
        g0, g1 = OWN[c]
        r = results[c]["res2_own"]
        res2[g0 * P:(g0 + 1) * P] = r[:P]
        res2[g1 * P:(g1 + 1) * P] = r[P:]
        rank = results[c]["rank_out"][0]
        sel = rank > 0.0
        slots = rank[sel].astype(np.int64) - 1
        moe[PERM_TOKENS[np.where(sel)[0]]] += results[c]["moe_cap"][slots]
    return moe.reshape(B, S, H), res2.reshape(B, S, H)
